# revision 22
# baseline (speedup 1.0000x reference)
"""GCN 3-layer message-passing kernel for TRN2 (8 NeuronCores, SPMD).

Self-contained: takes FULL inputs, shards internally, runs a Bass/Tile
kernel via run_bass_kernel_spmd on cores 0-7, gathers the full output.

v3 strategy (all HW-measured):
  - fp16 tables; grouped dma_gather calls (G=8 target blocks x 4 source
    chunks per layer) with slot-aligned equalized padding so num_idxs is
    a compile-time immediate.
  - GCN norm factored: dis[src] folded into the tables at transform
    evacuation (per-partition scale), dis[dst] applied at the relu
    evacuation; bias/dis[dst] pre-loaded into PSUM by the accumulation
    group's opening matmul. One-hot slot matrices are then pure 0/1 and
    built by a single IS_EQ tensor_tensor per gather call (broadcast AP).
  - log_softmax batched over the whole shard (single Exp / Ln passes) to
    avoid per-block ACT activation-table reloads (~1.3us each).
  - single_packet=False on the gathers (True wedges the device).
"""

import os
import sys

sys.path.insert(0, "/opt/trn_rl_repo")

import numpy as np

import concourse.bass as bass  # noqa: F401
import concourse.mybir as mybir
import concourse.tile as tile
from concourse import bacc
from concourse._compat import cdiv
from concourse.bass_utils import run_bass_kernel_spmd

F32 = mybir.dt.float32
F16 = mybir.dt.float16
I16 = mybir.dt.int16
AL = mybir.AluOpType
AF = mybir.ActivationFunctionType

NC = 8
P = 128
G = 8  # target blocks per gather group

LAST_EXEC_NS = None
LAST_SCOPES = None


def _cdiv_np(a, b):
    return (a + b - 1) // b


def _group_cumcount(grp: np.ndarray) -> np.ndarray:
    n = len(grp)
    if n == 0:
        return np.zeros(0, dtype=np.int64)
    is_new = np.ones(n, dtype=bool)
    is_new[1:] = grp[1:] != grp[:-1]
    idx = np.arange(n)
    start = np.maximum.accumulate(np.where(is_new, idx, 0))
    return idx - start


def _preprocess(edge_index: np.ndarray, n_nodes: int, chunk: int):
    N = n_nodes
    S = N // NC
    NB = cdiv(S, P)
    NG = _cdiv_np(NB, G)
    NQ = cdiv(N, chunk)

    src = np.concatenate([edge_index[0], np.arange(N, dtype=np.int64)])
    dst = np.concatenate([edge_index[1], np.arange(N, dtype=np.int64)])
    deg = np.bincount(dst, minlength=N).astype(np.float64)
    dis = (1.0 / np.sqrt(deg)).astype(np.float32)

    core = dst // S
    blk = (dst % S) // P
    tloc = (dst % S) % P
    grp = blk // G
    q = src // chunk
    sloc = src - q * chunk

    counts = np.zeros((NC, NB, NQ), dtype=np.int64)
    np.add.at(counts, (core, blk, q), 1)
    slots = np.maximum(1, _cdiv_np(counts.max(axis=0), P))  # [NB, NQ] shared

    kb = slots.sum(axis=1)
    K_total = int(kb.sum())
    IW = K_total * 8  # int16 idx columns (128 idxs -> 8 cols of 16)

    slots_gq = np.zeros((NG, NQ), dtype=np.int64)
    sec_off = np.zeros((NB, NQ), dtype=np.int64)  # position offset within call
    for g in range(NG):
        bs = range(g * G, min((g + 1) * G, NB))
        acc = np.zeros(NQ, dtype=np.int64)
        for b in bs:
            sec_off[b, :] = acc * P
            acc += slots[b, :]
        slots_gq[g, :] = acc
    MAXSL = int(slots_gq.max())

    ix_off = np.zeros((NG, NQ), dtype=np.int64)  # idx16 col offset per call
    glob_off = np.zeros((NG, NQ), dtype=np.int64)  # position offset per call
    tn_off = np.zeros((NG, NQ), dtype=np.int64)  # tn col offset per call
    acc = 0
    tacc = 0
    for g in range(NG):
        for qq in range(NQ):
            ix_off[g, qq] = acc // 16
            glob_off[g, qq] = acc
            tn_off[g, qq] = tacc
            acc += int(slots_gq[g, qq]) * P
            tacc += int(slots_gq[g, qq])
    assert acc == K_total * P and acc // 16 == IW and tacc == K_total

    order = np.lexsort((src, blk, q, grp, core))
    so_sloc = sloc[order]
    so_tl = tloc[order]
    so_core = core[order]
    so_blk = blk[order]
    so_q = q[order]
    so_grp = grp[order]

    per_core = []
    for c in range(NC):
        m = so_core == c
        cb, cq, cg = so_blk[m], so_q[m], so_grp[m]
        csl, ctl = so_sloc[m], so_tl[m]
        pos = _group_cumcount(cb * NQ + cq)  # position within (b, q)

        idx_flat = np.zeros(K_total * P, dtype=np.int16)  # dummies -> row 0
        gp = glob_off[cg, cq] + sec_off[cb, cq] + pos
        idx_flat[gp] = csl.astype(np.int16)
        idx16 = np.ascontiguousarray(idx_flat.reshape(-1, 16).T)  # [16, IW]

        tn = np.full((P, K_total), -1.0, dtype=np.float16)  # tv per call slot
        prow = pos % P
        pcol = tn_off[cg, cq] + (sec_off[cb, cq] + pos) // P
        tn[prow, pcol] = ctl.astype(np.float16)

        # per-node dis arranged [tloc, block]; unused rows -> 1.0
        disc = np.ones((P, NB), dtype=np.float32)
        nodes = np.arange(S)
        disc[nodes % P, nodes // P] = dis[c * S : (c + 1) * S]

        per_core.append(
            {"idx16": np.tile(idx16, (8, 1)), "tn": tn, "disc": disc,
             "dis": dis[c * S : (c + 1) * S]}
        )

    return {
        "slots": slots, "slots_gq": slots_gq, "sec_off": sec_off,
        "K_total": K_total, "IW": IW, "MAXSL": MAXSL,
        "ix_off": ix_off, "tn_off": tn_off,
        "NB": NB, "NG": NG, "NQ": NQ, "per_core": per_core,
    }


def _build_program(meta, n_nodes: int, chunk: int, fin, fh, fout):
    N = n_nodes
    S = N // NC
    NB = meta["NB"]
    NG = meta["NG"]
    NQ = meta["NQ"]
    NPAIR = (NB + 1) // 2
    slots = meta["slots"]
    slots_gq = meta["slots_gq"]
    sec_off = meta["sec_off"]
    K_total = meta["K_total"]
    IW = meta["IW"]
    MAXSL = meta["MAXSL"]
    ix_off = meta["ix_off"]
    tn_off = meta["tn_off"]
    fo_pad = 128  # t3 row padded to 128 fp16 = 256B (gather granularity)

    nc = bacc.Bacc()

    xT = nc.dram_tensor("xT", [fin, S], F16, kind="ExternalInput")
    W1 = nc.dram_tensor("W1", [fin, fh], F16, kind="ExternalInput")
    W2 = nc.dram_tensor("W2", [fh, fh], F16, kind="ExternalInput")
    W3 = nc.dram_tensor("W3", [fh, fout], F16, kind="ExternalInput")
    LW = nc.dram_tensor("LW", [2 * fh + fout, fout], F16, kind="ExternalInput")
    idx16 = nc.dram_tensor("idx16", [P, IW], I16, kind="ExternalInput")
    tn = nc.dram_tensor("tn", [P, K_total], F16, kind="ExternalInput")
    iotam_in = nc.dram_tensor("iotam", [P, MAXSL * P], F16, kind="ExternalInput")
    ident_in = nc.dram_tensor("ident", [P, P], F16, kind="ExternalInput")
    disc_in = nc.dram_tensor("disc", [P, NB], F32, kind="ExternalInput")
    pb1_in = nc.dram_tensor("pb1", [P, NPAIR * 2 * fh], F16, kind="ExternalInput")
    pb2_in = nc.dram_tensor("pb2", [P, NPAIR * 2 * fh], F16, kind="ExternalInput")
    pb3_in = nc.dram_tensor("pb3", [P, NPAIR * 2 * fh], F16, kind="ExternalInput")
    lbbc_in = nc.dram_tensor("lbbc", [P, fout], F16, kind="ExternalInput")
    out_sh = nc.dram_tensor("out_sh", [S, fout], F32, kind="ExternalOutput")

    t1_sh = nc.dram_tensor("t1_sh", [S, fh], F16)
    t2_sh = nc.dram_tensor("t2_sh", [S, fh], F16)
    t3_sh = nc.dram_tensor("t3_sh", [S, fo_pad], F16)
    t1_full = nc.dram_tensor("t1_full", [N, fh], F16, addr_space="Shared")
    t2_full = nc.dram_tensor("t2_full", [N, fh], F16, addr_space="Shared")
    t3_full = nc.dram_tensor("t3_full", [N, fo_pad], F16, addr_space="Shared")
    h1T_sh = nc.dram_tensor("h1T_sh", [fh, S], F16)
    h2T_sh = nc.dram_tensor("h2T_sh", [fh, S], F16)

    rg = [list(range(NC))]

    def used_rows(b):
        return min(P, S - b * P)

    with tile.TileContext(nc) as tc:
        with (
            tc.tile_pool(name="const", bufs=1) as cpool,
            tc.tile_pool(name="sb", bufs=3) as pool,
            tc.tile_pool(name="gath", bufs=2) as gpool,
            tc.tile_pool(name="ps", bufs=1, space="PSUM") as psp,
            tc.tile_pool(name="ps2p", bufs=2, space="PSUM") as psp2,
        ):
            iotam_t = cpool.tile([P, MAXSL * P], F16)
            nc.sync.dma_start(out=iotam_t[:], in_=iotam_in[:, :])
            ident_t = cpool.tile([P, P], F16)
            nc.sync.dma_start(out=ident_t[:], in_=ident_in[:, :])
            disc_t = cpool.tile([P, NB], F32)
            nc.sync.dma_start(out=disc_t[:], in_=disc_in[:, :])
            w1_t = cpool.tile([P, 2, fh], F16)
            nc.sync.dma_start(out=w1_t[:], in_=W1[:, :].rearrange("(c k) f -> k c f", k=P))
            w2_t = cpool.tile([P, 2, fh], F16)
            nc.sync.dma_start(out=w2_t[:], in_=W2[:, :].rearrange("(c k) f -> k c f", k=P))
            w3_t = cpool.tile([P, 2, fout], F16)
            nc.sync.dma_start(out=w3_t[:], in_=W3[:, :].rearrange("(c k) f -> k c f", k=P))
            lw12_t = cpool.tile([P, 4, fout], F16)
            nc.sync.dma_start(
                out=lw12_t[:], in_=LW[: 4 * P, :].rearrange("(c k) f -> k c f", k=P)
            )
            lw3_t = cpool.tile([fout, fout], F16)
            nc.sync.dma_start(out=lw3_t[:], in_=LW[4 * P :, :])
            lbbc = cpool.tile([P, fout], F16)
            nc.sync.dma_start(out=lbbc[:], in_=lbbc_in[:, :])
            logits = cpool.tile([P, NB, fout], F32)
            work1 = cpool.tile([P, NB, fout], F32)
            work2 = cpool.tile([P, NB, fout], F32)

            sc_T = nc.enter_named_scope("phaseT", False)
            for b in range(NB):
                u = used_rows(b)
                ps1 = psp2.tile([P, fh], F32, tag="ps2")
                for cc in range(2):
                    xt = pool.tile([P, P], F16, tag="xt")
                    nc.sync.dma_start(
                        out=xt[:, :u], in_=xT[cc * P : (cc + 1) * P, b * P : b * P + u]
                    )
                    nc.tensor.matmul(
                        ps1[:u, :],
                        lhsT=xt[:, :u],
                        rhs=w1_t[:, cc, :],
                        start=(cc == 0),
                        stop=(cc == 1),
                    )
                ev = pool.tile([P, fh], F16, tag="ev")
                nc.vector.tensor_scalar(
                    out=ev[:u, :], in0=ps1[:u, :], scalar1=disc_t[:u, b : b + 1],
                    scalar2=None, op0=AL.mult,
                )
                nc.sync.dma_start(out=t1_sh[b * P : b * P + u, :], in_=ev[:u, :])

            nc.leave_named_scope("phaseT", sc_T[0], False)
            sc = nc.enter_named_scope("ag1", False)
            nc.gpsimd.collective_compute(
                "AllGather", AL.bypass, ins=[t1_sh[:, :]], outs=[t1_full[:, :]],
                replica_groups=rg,
            )
            nc.leave_named_scope("ag1", sc[0], False)

            def layer(li, table, felem, wnext_t, fnext, pb_in, tnext_sh, hT_sh_,
                      fnext_pad=None):
                fagg = fh if li < 3 else fout
                for g in range(NG):
                    b_lo = g * G
                    b_hi = min((g + 1) * G, NB)
                    nb_g = b_hi - b_lo
                    npair_g = (nb_g + 1) // 2

                    psabs = []
                    for i in range(npair_g):
                        pbt = pool.tile([P, 2 * fh], F16, tag="pbt", name=f"pbt{i}")
                        po = (b_lo // 2 + i) * 2 * fh
                        nc.sync.dma_start(
                            out=pbt[:], in_=pb_in[:, po : po + 2 * fh]
                        )
                        pt = psp.tile([P, 2 * fh], F32, tag=f"psab{i}",
                                      name=f"psab{i}")
                        nc.tensor.matmul(
                            pt[:], lhsT=ident_t[:], rhs=pbt[:],
                            start=True, stop=False,
                        )
                        psabs.append(pt)
                    psas = [
                        psabs[i // 2][:, (i % 2) * fh : (i % 2 + 1) * fh]
                        for i in range(nb_g)
                    ]

                    for qq in range(NQ):
                        sl_gq = int(slots_gq[g, qq])
                        nidx = sl_gq * P
                        ixt = pool.tile([P, MAXSL * 8], I16, tag="ixt")
                        io = int(ix_off[g, qq])
                        nc.sync.dma_start(
                            out=ixt[:, : sl_gq * 8], in_=idx16[:, io : io + sl_gq * 8]
                        )
                        tnc = pool.tile([P, MAXSL], F16, tag="tnc")
                        tno = int(tn_off[g, qq])
                        nc.sync.dma_start(
                            out=tnc[:, :sl_gq], in_=tn[:, tno : tno + sl_gq]
                        )
                        dst = gpool.tile([P, MAXSL, felem], F16, tag="dst")
                        base = qq * chunk
                        rows = min(chunk, N - base)
                        nc.gpsimd.dma_gather(
                            dst[:, :sl_gq, :],
                            table[base : base + rows, :],
                            ixt[:, : sl_gq * 8],
                            nidx,
                            nidx,
                            felem,
                            single_packet=False,
                        )
                        stb = gpool.tile([P, MAXSL, P], F16, tag="stb")
                        tv_b = tnc[:, 0:sl_gq, None].broadcast_to([P, sl_gq, P])
                        nc.vector.tensor_tensor(
                            out=stb[:, :sl_gq, :],
                            in0=iotam_t[:, : sl_gq * P].rearrange(
                                "p (s c) -> p s c", c=P
                            ),
                            in1=tv_b,
                            op=AL.is_equal,
                        )
                        for bi in range(nb_g):
                            b = b_lo + bi
                            col0 = int(sec_off[b, qq]) // P
                            closer = (bi % 2 == 1) or (bi == nb_g - 1)
                            nsl = int(slots[b, qq])
                            for j in range(nsl):
                                s = col0 + j
                                last = qq == NQ - 1 and closer and j == nsl - 1
                                nc.tensor.matmul(
                                    psas[bi][:, :fagg],
                                    lhsT=stb[:, s, :],
                                    rhs=dst[:, s, :fagg],
                                    start=False,
                                    stop=last,
                                )

                    for bi in range(nb_g):
                        b = b_lo + bi
                        u = used_rows(b)
                        psa = psas[bi]
                        h_sb = pool.tile([P, fagg], F16, tag="h_sb")
                        nc.scalar.activation(
                            h_sb[:], psa[:, :fagg], AF.Relu,
                            scale=disc_t[:, b : b + 1],
                        )

                        if li < 3:
                            ps2 = psp2.tile([P, fnext], F32, tag="ps2")
                            for cc in range(2):
                                pst = psp2.tile([P, P], F16, tag="pst")
                                nc.tensor.transpose(
                                    pst[:], h_sb[:, cc * P : (cc + 1) * P], ident_t[:]
                                )
                                hT = pool.tile([P, P], F16, tag="hT")
                                nc.vector.tensor_copy(hT[:], pst[:])
                                nc.sync.dma_start(
                                    out=hT_sh_[cc * P : (cc + 1) * P, b * P : b * P + u],
                                    in_=hT[:, :u],
                                )
                                nc.tensor.matmul(
                                    ps2[:u, :],
                                    lhsT=hT[:, :u],
                                    rhs=wnext_t[:, cc, :fnext],
                                    start=(cc == 0),
                                    stop=(cc == 1),
                                )
                            fp = fnext_pad or fnext
                            ev2 = pool.tile([P, fp], F16, tag="ev")
                            nc.vector.tensor_scalar(
                                out=ev2[:u, :fnext], in0=ps2[:u, :],
                                scalar1=disc_t[:u, b : b + 1],
                                scalar2=None, op0=AL.mult,
                            )
                            if fp > fnext:
                                nc.vector.memset(ev2[:u, fnext:fp], 0.0)
                            nc.sync.dma_start(
                                out=tnext_sh[b * P : b * P + u, :fp],
                                in_=ev2[:u, :fp],
                            )
                        else:
                            ps3t = psp2.tile([fout, P], F16, tag="pst")
                            nc.tensor.transpose(ps3t[:], h_sb[:, :fout], ident_t[:])
                            h3T = pool.tile([fout, P], F16, tag="hT")
                            nc.vector.tensor_copy(h3T[:], ps3t[:])
                            pso = psp2.tile([P, fout], F32, tag="ps2")
                            for cc in range(2):
                                r1 = pool.tile([P, P], F16, tag=f"rl{cc}")
                                nc.sync.dma_start(
                                    out=r1[:, :u],
                                    in_=h1T_sh[cc * P : (cc + 1) * P, b * P : b * P + u],
                                )
                                nc.tensor.matmul(
                                    pso[:u, :], lhsT=r1[:, :u], rhs=lw12_t[:, cc, :],
                                    start=(cc == 0), stop=False,
                                )
                            for cc in range(2):
                                r2 = pool.tile([P, P], F16, tag=f"rl{2 + cc}")
                                nc.sync.dma_start(
                                    out=r2[:, :u],
                                    in_=h2T_sh[cc * P : (cc + 1) * P, b * P : b * P + u],
                                )
                                nc.tensor.matmul(
                                    pso[:u, :], lhsT=r2[:, :u], rhs=lw12_t[:, 2 + cc, :],
                                    start=False, stop=False,
                                )
                            nc.tensor.matmul(
                                pso[:u, :], lhsT=h3T[:, :u], rhs=lw3_t[:, :],
                                start=False, stop=False,
                            )
                            nc.tensor.matmul(
                                pso[:u, :], lhsT=ident_t[:, :u], rhs=lbbc[:, :],
                                start=False, stop=True,
                            )
                            nc.vector.tensor_copy(logits[:, b, :], pso[:, :])

            sc = nc.enter_named_scope("L1", False)
            layer(1, t1_full, fh, w2_t, fh, pb1_in, t2_sh, h1T_sh)
            nc.leave_named_scope("L1", sc[0], False)
            sc = nc.enter_named_scope("ag2", False)
            nc.gpsimd.collective_compute(
                "AllGather", AL.bypass, ins=[t2_sh[:, :]], outs=[t2_full[:, :]],
                replica_groups=rg,
            )
            nc.leave_named_scope("ag2", sc[0], False)
            sc = nc.enter_named_scope("L2", False)
            layer(2, t2_full, fh, w3_t, fout, pb2_in, t3_sh, h2T_sh,
                  fnext_pad=fo_pad)
            nc.leave_named_scope("L2", sc[0], False)
            sc = nc.enter_named_scope("ag3", False)
            nc.gpsimd.collective_compute(
                "AllGather", AL.bypass, ins=[t3_sh[:, :]], outs=[t3_full[:, :]],
                replica_groups=rg,
            )
            nc.leave_named_scope("ag3", sc[0], False)
            sc = nc.enter_named_scope("L3", False)
            layer(3, t3_full, fo_pad, None, None, pb3_in, None, None)

            # batched log_softmax over the whole shard
            mx = pool.tile([P, NB], F32, tag="mx")
            nc.vector.tensor_reduce(
                mx[:], logits[:, :, :], mybir.AxisListType.X, AL.max
            )
            nc.vector.tensor_tensor(
                out=work1[:],
                in0=logits[:, :, :],
                in1=mx[:, :, None].broadcast_to([P, NB, fout]),
                op=AL.subtract,
            )
            nc.scalar.activation(
                work2[:].rearrange("p a b -> p (a b)"),
                work1[:].rearrange("p a b -> p (a b)"),
                AF.Exp,
            )
            ssum = pool.tile([P, NB], F32, tag="ssum")
            nc.vector.tensor_reduce(
                ssum[:], work2[:, :, :], mybir.AxisListType.X, AL.add
            )
            ls = pool.tile([P, NB], F32, tag="ls")
            nc.scalar.activation(ls[:], ssum[:], AF.Ln)
            mls = pool.tile([P, NB], F32, tag="mls")
            nc.vector.tensor_tensor(out=mls[:], in0=mx[:], in1=ls[:], op=AL.add)
            nc.vector.tensor_tensor(
                out=work1[:],
                in0=logits[:, :, :],
                in1=mls[:, :, None].broadcast_to([P, NB, fout]),
                op=AL.subtract,
            )
            for b in range(NB):
                u = used_rows(b)
                nc.sync.dma_start(
                    out=out_sh[b * P : b * P + u, :], in_=work1[:u, b, :]
                )
            nc.leave_named_scope("L3", sc[0], False)

    nc.finalize()
    return nc


def _prepare(x, edge_index, W1, b1, W2, b2, W3, b3, lin_w, lin_b, chunk):
    x = np.asarray(x)
    N = x.shape[0]
    S = N // NC
    fin, fh, fout = W1.shape[0], W2.shape[0], W3.shape[1]
    NB = cdiv(S, P)
    NPAIR = (NB + 1) // 2

    meta = _preprocess(np.asarray(edge_index, dtype=np.int64), N, chunk)
    nc = _build_program(meta, N, chunk, fin, fh, fout)

    iotam = np.tile(
        np.tile(np.arange(P, dtype=np.float16), meta["MAXSL"]), (P, 1)
    )
    ident = np.eye(P, dtype=np.float16)
    lbbc = np.tile(np.asarray(lin_b, np.float16), (P, 1))

    in_maps = []
    for c in range(NC):
        xs = np.asarray(x[c * S : (c + 1) * S], np.float16)
        pc = meta["per_core"][c]
        dis_c = pc["dis"]  # [S] f32

        def pbias(bvec, f):
            # [P, NPAIR, 2*fh]: bias/dis[node] at (tloc, pair, half*fh+feat)
            pb = np.zeros((P, NPAIR, 2 * fh), dtype=np.float32)
            for b in range(NB):
                u = min(P, S - b * P)
                nodes = np.arange(b * P, b * P + u)
                col0 = (b % 2) * fh
                pb[:u, b // 2, col0 : col0 + f] = (
                    bvec[None, :f] / dis_c[nodes][:, None]
                )
            return pb.reshape(P, NPAIR * 2 * fh).astype(np.float16)

        in_maps.append(
            {
                "xT": np.ascontiguousarray(xs.T),
                "W1": np.asarray(W1, np.float16),
                "W2": np.asarray(W2, np.float16),
                "W3": np.asarray(W3, np.float16),
                "LW": np.asarray(lin_w, np.float16),
                "idx16": pc["idx16"],
                "tn": pc["tn"],
                "iotam": iotam,
                "ident": ident,
                "disc": pc["disc"],
                "pb1": pbias(np.asarray(b1, np.float32), fh),
                "pb2": pbias(np.asarray(b2, np.float32), fh),
                "pb3": pbias(np.asarray(b3, np.float32), fout),
                "lbbc": lbbc,
            }
        )
    return nc, in_maps


def kernel(x, edge_index, W1, b1, W2, b2, W3, b3, lin_w, lin_b):
    global LAST_EXEC_NS, LAST_SCOPES
    nc, in_maps = _prepare(
        x, edge_index, W1, b1, W2, b2, W3, b3, lin_w, lin_b, chunk=25000
    )
    trace = bool(os.environ.get("GCN_TRACE"))
    res = run_bass_kernel_spmd(nc, in_maps, list(range(NC)), trace=trace)
    LAST_EXEC_NS = res.exec_time_ns
    LAST_SCOPES = res.per_core_scope_times
    out = np.concatenate([res.results[c]["out_sh"] for c in range(NC)], axis=0)
    return out.astype(np.float32)


# revision 25
# speedup vs baseline: 1.4569x; 1.4569x over previous
"""GCN 3-layer message-passing kernel for TRN2 (8 NeuronCores, SPMD).

Self-contained: takes FULL inputs, shards internally, runs a Bass/Tile
kernel via run_bass_kernel_spmd on cores 0-7, gathers the full output.

v3 strategy (all HW-measured):
  - fp16 tables; grouped dma_gather calls (G=8 target blocks x 4 source
    chunks per layer) with slot-aligned equalized padding so num_idxs is
    a compile-time immediate.
  - GCN norm factored: dis[src] folded into the tables at transform
    evacuation (per-partition scale), dis[dst] applied at the relu
    evacuation; bias/dis[dst] pre-loaded into PSUM by the accumulation
    group's opening matmul. One-hot slot matrices are then pure 0/1 and
    built by a single IS_EQ tensor_tensor per gather call (broadcast AP).
  - log_softmax batched over the whole shard (single Exp / Ln passes) to
    avoid per-block ACT activation-table reloads (~1.3us each).
  - single_packet=False on the gathers (True wedges the device).
"""

import os
import sys

sys.path.insert(0, "/opt/trn_rl_repo")

import numpy as np

import concourse.bass as bass  # noqa: F401
import concourse.mybir as mybir
import concourse.tile as tile
from concourse import bacc
from concourse._compat import cdiv
from concourse.bass_utils import run_bass_kernel_spmd

F32 = mybir.dt.float32
F16 = mybir.dt.float16
I16 = mybir.dt.int16
AL = mybir.AluOpType
AF = mybir.ActivationFunctionType

NC = 8
P = 128
G = 8  # target blocks per gather group

LAST_EXEC_NS = None
LAST_SCOPES = None


def _cdiv_np(a, b):
    return (a + b - 1) // b


def _group_cumcount(grp: np.ndarray) -> np.ndarray:
    n = len(grp)
    if n == 0:
        return np.zeros(0, dtype=np.int64)
    is_new = np.ones(n, dtype=bool)
    is_new[1:] = grp[1:] != grp[:-1]
    idx = np.arange(n)
    start = np.maximum.accumulate(np.where(is_new, idx, 0))
    return idx - start


def _preprocess(edge_index: np.ndarray, n_nodes: int, chunk: int):
    N = n_nodes
    S = N // NC
    NB = cdiv(S, P)
    NG = _cdiv_np(NB, G)
    NQ = 4  # source stripes (one collective each)
    SB = _cdiv_np(NB, NQ)  # blocks per stripe
    srows = [min(SB * P, S - k * SB * P) for k in range(NQ)]
    s_off = np.zeros(NQ, dtype=np.int64)  # stripe start row within a shard

    src = np.concatenate([edge_index[0], np.arange(N, dtype=np.int64)])
    dst = np.concatenate([edge_index[1], np.arange(N, dtype=np.int64)])
    deg = np.bincount(dst, minlength=N).astype(np.float64)
    dis = (1.0 / np.sqrt(deg)).astype(np.float32)

    core = dst // S
    blk = (dst % S) // P
    tloc = (dst % S) % P
    grp = blk // G
    # source stripe mapping: stripe k holds rows [k*SB*P, ...) of every
    # shard; the gathered table t_full{k} is core-major within the stripe
    s_core = src // S
    s_r = src % S
    q = np.minimum(s_r // (SB * P), NQ - 1)
    srows_np = np.array(srows, dtype=np.int64)
    sloc = s_core * srows_np[q] + (s_r - q * SB * P)

    counts = np.zeros((NC, NB, NQ), dtype=np.int64)
    np.add.at(counts, (core, blk, q), 1)
    slots = np.maximum(1, _cdiv_np(counts.max(axis=0), P))  # [NB, NQ] shared

    kb = slots.sum(axis=1)
    K_total = int(kb.sum())
    IW = K_total * 8  # int16 idx columns (128 idxs -> 8 cols of 16)

    slots_gq = np.zeros((NG, NQ), dtype=np.int64)
    sec_off = np.zeros((NB, NQ), dtype=np.int64)  # position offset within call
    for g in range(NG):
        bs = range(g * G, min((g + 1) * G, NB))
        acc = np.zeros(NQ, dtype=np.int64)
        for b in bs:
            sec_off[b, :] = acc * P
            acc += slots[b, :]
        slots_gq[g, :] = acc
    MAXSL = int(slots_gq.max())

    ix_off = np.zeros((NG, NQ), dtype=np.int64)  # idx16 col offset per call
    glob_off = np.zeros((NG, NQ), dtype=np.int64)  # position offset per call
    tn_off = np.zeros((NG, NQ), dtype=np.int64)  # tn col offset per call
    acc = 0
    tacc = 0
    for g in range(NG):
        for qq in range(NQ):
            ix_off[g, qq] = acc // 16
            glob_off[g, qq] = acc
            tn_off[g, qq] = tacc
            acc += int(slots_gq[g, qq]) * P
            tacc += int(slots_gq[g, qq])
    assert acc == K_total * P and acc // 16 == IW and tacc == K_total

    order = np.lexsort((src, blk, q, grp, core))
    so_sloc = sloc[order]
    so_tl = tloc[order]
    so_core = core[order]
    so_blk = blk[order]
    so_q = q[order]
    so_grp = grp[order]

    per_core = []
    for c in range(NC):
        m = so_core == c
        cb, cq, cg = so_blk[m], so_q[m], so_grp[m]
        csl, ctl = so_sloc[m], so_tl[m]
        pos = _group_cumcount(cb * NQ + cq)  # position within (b, q)

        idx_flat = np.zeros(K_total * P, dtype=np.int16)  # dummies -> row 0
        gp = glob_off[cg, cq] + sec_off[cb, cq] + pos
        idx_flat[gp] = csl.astype(np.int16)
        idx16 = np.ascontiguousarray(idx_flat.reshape(-1, 16).T)  # [16, IW]

        tn = np.full((P, K_total), -1.0, dtype=np.float16)  # tv per call slot
        prow = pos % P
        pcol = tn_off[cg, cq] + (sec_off[cb, cq] + pos) // P
        tn[prow, pcol] = ctl.astype(np.float16)

        # per-node dis arranged [tloc, block]; unused rows -> 1.0
        disc = np.ones((P, NB), dtype=np.float32)
        nodes = np.arange(S)
        disc[nodes % P, nodes // P] = dis[c * S : (c + 1) * S]

        per_core.append(
            {"idx16": np.tile(idx16, (8, 1)), "tn": tn, "disc": disc,
             "dis": dis[c * S : (c + 1) * S]}
        )

    return {
        "slots": slots, "slots_gq": slots_gq, "sec_off": sec_off,
        "K_total": K_total, "IW": IW, "MAXSL": MAXSL,
        "ix_off": ix_off, "tn_off": tn_off, "SB": SB,
        "NB": NB, "NG": NG, "NQ": NQ, "per_core": per_core,
    }


def _build_program(meta, n_nodes: int, chunk: int, fin, fh, fout):
    N = n_nodes
    S = N // NC
    NB = meta["NB"]
    NG = meta["NG"]
    NQ = meta["NQ"]
    NPAIR = (NB + 1) // 2
    slots = meta["slots"]
    slots_gq = meta["slots_gq"]
    sec_off = meta["sec_off"]
    K_total = meta["K_total"]
    IW = meta["IW"]
    MAXSL = meta["MAXSL"]
    ix_off = meta["ix_off"]
    tn_off = meta["tn_off"]
    fo_pad = 128  # t3 row padded to 128 fp16 = 256B (gather granularity)

    nc = bacc.Bacc()

    xT = nc.dram_tensor("xT", [fin, S], F16, kind="ExternalInput")
    W1 = nc.dram_tensor("W1", [fin, fh], F16, kind="ExternalInput")
    W2 = nc.dram_tensor("W2", [fh, fh], F16, kind="ExternalInput")
    W3 = nc.dram_tensor("W3", [fh, fout], F16, kind="ExternalInput")
    LW = nc.dram_tensor("LW", [2 * fh + fout, fout], F16, kind="ExternalInput")
    idx16 = nc.dram_tensor("idx16", [P, IW], I16, kind="ExternalInput")
    tn = nc.dram_tensor("tn", [P, K_total], F16, kind="ExternalInput")
    iotam_in = nc.dram_tensor("iotam", [P, MAXSL * P], F16, kind="ExternalInput")
    ident_in = nc.dram_tensor("ident", [P, P], F16, kind="ExternalInput")
    disc_in = nc.dram_tensor("disc", [P, NB], F32, kind="ExternalInput")
    pb1_in = nc.dram_tensor("pb1", [P, NPAIR * 2 * fh], F16, kind="ExternalInput")
    pb2_in = nc.dram_tensor("pb2", [P, NPAIR * 2 * fh], F16, kind="ExternalInput")
    pb3_in = nc.dram_tensor("pb3", [P, NPAIR * 2 * fh], F16, kind="ExternalInput")
    lbbc_in = nc.dram_tensor("lbbc", [P, fout], F16, kind="ExternalInput")
    out_sh = nc.dram_tensor("out_sh", [S, fout], F32, kind="ExternalOutput")

    SB = meta["SB"]
    KS = NQ
    srows = [min(SB * P, S - k * SB * P) for k in range(KS)]
    t1_shs = [nc.dram_tensor(f"t1_sh{k}", [srows[k], fh], F16) for k in range(KS)]
    t2_shs = [nc.dram_tensor(f"t2_sh{k}", [srows[k], fh], F16) for k in range(KS)]
    t3_shs = [
        nc.dram_tensor(f"t3_sh{k}", [srows[k], fo_pad], F16) for k in range(KS)
    ]
    t1_fulls = [
        nc.dram_tensor(f"t1_full{k}", [NC * srows[k], fh], F16,
                       addr_space="Shared")
        for k in range(KS)
    ]
    t2_fulls = [
        nc.dram_tensor(f"t2_full{k}", [NC * srows[k], fh], F16,
                       addr_space="Shared")
        for k in range(KS)
    ]
    t3_fulls = [
        nc.dram_tensor(f"t3_full{k}", [NC * srows[k], fo_pad], F16,
                       addr_space="Shared")
        for k in range(KS)
    ]
    h1T_sh = nc.dram_tensor("h1T_sh", [fh, S], F16)
    h2T_sh = nc.dram_tensor("h2T_sh", [fh, S], F16)

    def stripe_slice(b):
        k = min(b // SB, KS - 1)
        r0 = (b - k * SB) * P
        return k, r0

    rg = [list(range(NC))]

    def used_rows(b):
        return min(P, S - b * P)

    with tile.TileContext(nc) as tc:
        with (
            tc.tile_pool(name="const", bufs=1) as cpool,
            tc.tile_pool(name="sb", bufs=3) as pool,
            tc.tile_pool(name="gath", bufs=2) as gpool,
            tc.tile_pool(name="ps", bufs=1, space="PSUM") as psp,
            tc.tile_pool(name="ps2p", bufs=2, space="PSUM") as psp2,
        ):
            iotam_t = cpool.tile([P, MAXSL * P], F16)
            nc.sync.dma_start(out=iotam_t[:], in_=iotam_in[:, :])
            ident_t = cpool.tile([P, P], F16)
            nc.sync.dma_start(out=ident_t[:], in_=ident_in[:, :])
            disc_t = cpool.tile([P, NB], F32)
            nc.sync.dma_start(out=disc_t[:], in_=disc_in[:, :])
            w1_t = cpool.tile([P, 2, fh], F16)
            nc.sync.dma_start(out=w1_t[:], in_=W1[:, :].rearrange("(c k) f -> k c f", k=P))
            w2_t = cpool.tile([P, 2, fh], F16)
            nc.sync.dma_start(out=w2_t[:], in_=W2[:, :].rearrange("(c k) f -> k c f", k=P))
            w3_t = cpool.tile([P, 2, fout], F16)
            nc.sync.dma_start(out=w3_t[:], in_=W3[:, :].rearrange("(c k) f -> k c f", k=P))
            lw12_t = cpool.tile([P, 4, fout], F16)
            nc.sync.dma_start(
                out=lw12_t[:], in_=LW[: 4 * P, :].rearrange("(c k) f -> k c f", k=P)
            )
            lw3_t = cpool.tile([fout, fout], F16)
            nc.sync.dma_start(out=lw3_t[:], in_=LW[4 * P :, :])
            lbbc = cpool.tile([P, fout], F16)
            nc.sync.dma_start(out=lbbc[:], in_=lbbc_in[:, :])
            logits = cpool.tile([P, NB, fout], F32)
            work1 = cpool.tile([P, NB, fout], F32)
            work2 = cpool.tile([P, NB, fout], F32)

            sc_T = nc.enter_named_scope("phaseT", False)
            for b in range(NB):
                u = used_rows(b)
                ps1 = psp2.tile([P, fh], F32, tag="ps2")
                for cc in range(2):
                    xt = pool.tile([P, P], F16, tag="xt")
                    nc.sync.dma_start(
                        out=xt[:, :u], in_=xT[cc * P : (cc + 1) * P, b * P : b * P + u]
                    )
                    nc.tensor.matmul(
                        ps1[:u, :],
                        lhsT=xt[:, :u],
                        rhs=w1_t[:, cc, :],
                        start=(cc == 0),
                        stop=(cc == 1),
                    )
                ev = pool.tile([P, fh], F16, tag="ev")
                nc.vector.tensor_scalar(
                    out=ev[:u, :], in0=ps1[:u, :], scalar1=disc_t[:u, b : b + 1],
                    scalar2=None, op0=AL.mult,
                )
                k, r0 = stripe_slice(b)
                nc.sync.dma_start(out=t1_shs[k][r0 : r0 + u, :], in_=ev[:u, :])
                if b == min((k + 1) * SB, NB) - 1:
                    nc.gpsimd.collective_compute(
                        "AllGather", AL.bypass, ins=[t1_shs[k][:, :]],
                        outs=[t1_fulls[k][:, :]], replica_groups=rg,
                    )

            nc.leave_named_scope("phaseT", sc_T[0], False)

            def layer(li, tables, felem, wnext_t, fnext, pb_in, tnext_shs,
                      tnext_fulls, hT_sh_, fnext_pad=None):
                fagg = fh if li < 3 else fout
                for g in range(NG):
                    b_lo = g * G
                    b_hi = min((g + 1) * G, NB)
                    nb_g = b_hi - b_lo
                    npair_g = (nb_g + 1) // 2

                    psabs = []
                    for i in range(npair_g):
                        pbt = pool.tile([P, 2 * fh], F16, tag="pbt", name=f"pbt{i}")
                        po = (b_lo // 2 + i) * 2 * fh
                        nc.scalar.dma_start(
                            out=pbt[:], in_=pb_in[:, po : po + 2 * fh]
                        )
                        pt = psp.tile([P, 2 * fh], F32, tag=f"psab{i}",
                                      name=f"psab{i}")
                        nc.tensor.matmul(
                            pt[:], lhsT=ident_t[:], rhs=pbt[:],
                            start=True, stop=False,
                        )
                        psabs.append(pt)
                    psas = [
                        psabs[i // 2][:, (i % 2) * fh : (i % 2 + 1) * fh]
                        for i in range(nb_g)
                    ]

                    for qq in range(NQ):
                        sl_gq = int(slots_gq[g, qq])
                        nidx = sl_gq * P
                        ixt = pool.tile([P, MAXSL * 8], I16, tag="ixt")
                        io = int(ix_off[g, qq])
                        nc.scalar.dma_start(
                            out=ixt[:, : sl_gq * 8], in_=idx16[:, io : io + sl_gq * 8]
                        )
                        tnc = pool.tile([P, MAXSL], F16, tag="tnc")
                        tno = int(tn_off[g, qq])
                        nc.scalar.dma_start(
                            out=tnc[:, :sl_gq], in_=tn[:, tno : tno + sl_gq]
                        )
                        dst = gpool.tile([P, MAXSL, felem], F16, tag="dst")
                        rows = NC * srows[qq]
                        nc.gpsimd.dma_gather(
                            dst[:, :sl_gq, :],
                            tables[qq][0:rows, :],
                            ixt[:, : sl_gq * 8],
                            nidx,
                            nidx,
                            felem,
                            single_packet=False,
                        )
                        stb = gpool.tile([P, MAXSL, P], F16, tag="stb")
                        tv_b = tnc[:, 0:sl_gq, None].broadcast_to([P, sl_gq, P])
                        nc.vector.tensor_tensor(
                            out=stb[:, :sl_gq, :],
                            in0=iotam_t[:, : sl_gq * P].rearrange(
                                "p (s c) -> p s c", c=P
                            ),
                            in1=tv_b,
                            op=AL.is_equal,
                        )
                        for bi in range(nb_g):
                            b = b_lo + bi
                            col0 = int(sec_off[b, qq]) // P
                            closer = (bi % 2 == 1) or (bi == nb_g - 1)
                            nsl = int(slots[b, qq])
                            for j in range(nsl):
                                s = col0 + j
                                last = qq == NQ - 1 and closer and j == nsl - 1
                                nc.tensor.matmul(
                                    psas[bi][:, :fagg],
                                    lhsT=stb[:, s, :],
                                    rhs=dst[:, s, :fagg],
                                    start=False,
                                    stop=last,
                                )

                    for bi in range(nb_g):
                        b = b_lo + bi
                        u = used_rows(b)
                        psa = psas[bi]
                        h_sb = pool.tile([P, fagg], F16, tag="h_sb")
                        nc.scalar.activation(
                            h_sb[:], psa[:, :fagg], AF.Relu,
                            scale=disc_t[:, b : b + 1],
                        )

                        if li < 3:
                            ps2 = psp2.tile([P, fnext], F32, tag="ps2")
                            for cc in range(2):
                                pst = psp2.tile([P, P], F16, tag="pst")
                                nc.tensor.transpose(
                                    pst[:], h_sb[:, cc * P : (cc + 1) * P], ident_t[:]
                                )
                                hT = pool.tile([P, P], F16, tag="hT")
                                nc.vector.tensor_copy(hT[:], pst[:])
                                nc.sync.dma_start(
                                    out=hT_sh_[cc * P : (cc + 1) * P, b * P : b * P + u],
                                    in_=hT[:, :u],
                                )
                                nc.tensor.matmul(
                                    ps2[:u, :],
                                    lhsT=hT[:, :u],
                                    rhs=wnext_t[:, cc, :fnext],
                                    start=(cc == 0),
                                    stop=(cc == 1),
                                )
                            fp = fnext_pad or fnext
                            ev2 = pool.tile([P, fp], F16, tag="ev")
                            nc.vector.tensor_scalar(
                                out=ev2[:u, :fnext], in0=ps2[:u, :],
                                scalar1=disc_t[:u, b : b + 1],
                                scalar2=None, op0=AL.mult,
                            )
                            if fp > fnext:
                                nc.vector.memset(ev2[:u, fnext:fp], 0.0)
                            k, r0 = stripe_slice(b)
                            nc.sync.dma_start(
                                out=tnext_shs[k][r0 : r0 + u, :fp],
                                in_=ev2[:u, :fp],
                            )
                            if b == min((k + 1) * SB, NB) - 1:
                                nc.gpsimd.collective_compute(
                                    "AllGather", AL.bypass,
                                    ins=[tnext_shs[k][:, :]],
                                    outs=[tnext_fulls[k][:, :]],
                                    replica_groups=rg,
                                )
                        else:
                            ps3t = psp2.tile([fout, P], F16, tag="pst")
                            nc.tensor.transpose(ps3t[:], h_sb[:, :fout], ident_t[:])
                            h3T = pool.tile([fout, P], F16, tag="hT")
                            nc.vector.tensor_copy(h3T[:], ps3t[:])
                            pso = psp2.tile([P, fout], F32, tag="ps2")
                            for cc in range(2):
                                r1 = pool.tile([P, P], F16, tag=f"rl{cc}")
                                nc.sync.dma_start(
                                    out=r1[:, :u],
                                    in_=h1T_sh[cc * P : (cc + 1) * P, b * P : b * P + u],
                                )
                                nc.tensor.matmul(
                                    pso[:u, :], lhsT=r1[:, :u], rhs=lw12_t[:, cc, :],
                                    start=(cc == 0), stop=False,
                                )
                            for cc in range(2):
                                r2 = pool.tile([P, P], F16, tag=f"rl{2 + cc}")
                                nc.sync.dma_start(
                                    out=r2[:, :u],
                                    in_=h2T_sh[cc * P : (cc + 1) * P, b * P : b * P + u],
                                )
                                nc.tensor.matmul(
                                    pso[:u, :], lhsT=r2[:, :u], rhs=lw12_t[:, 2 + cc, :],
                                    start=False, stop=False,
                                )
                            nc.tensor.matmul(
                                pso[:u, :], lhsT=h3T[:, :u], rhs=lw3_t[:, :],
                                start=False, stop=False,
                            )
                            nc.tensor.matmul(
                                pso[:u, :], lhsT=ident_t[:, :u], rhs=lbbc[:, :],
                                start=False, stop=True,
                            )
                            nc.vector.tensor_copy(logits[:, b, :], pso[:, :])

            sc = nc.enter_named_scope("L1", False)
            layer(1, t1_fulls, fh, w2_t, fh, pb1_in, t2_shs, t2_fulls, h1T_sh)
            nc.leave_named_scope("L1", sc[0], False)
            sc = nc.enter_named_scope("L2", False)
            layer(2, t2_fulls, fh, w3_t, fout, pb2_in, t3_shs, t3_fulls,
                  h2T_sh, fnext_pad=fo_pad)
            nc.leave_named_scope("L2", sc[0], False)
            sc = nc.enter_named_scope("L3", False)
            layer(3, t3_fulls, fo_pad, None, None, pb3_in, None, None, None)

            # batched log_softmax over the whole shard
            mx = pool.tile([P, NB], F32, tag="mx")
            nc.vector.tensor_reduce(
                mx[:], logits[:, :, :], mybir.AxisListType.X, AL.max
            )
            nc.vector.tensor_tensor(
                out=work1[:],
                in0=logits[:, :, :],
                in1=mx[:, :, None].broadcast_to([P, NB, fout]),
                op=AL.subtract,
            )
            nc.scalar.activation(
                work2[:].rearrange("p a b -> p (a b)"),
                work1[:].rearrange("p a b -> p (a b)"),
                AF.Exp,
            )
            ssum = pool.tile([P, NB], F32, tag="ssum")
            nc.vector.tensor_reduce(
                ssum[:], work2[:, :, :], mybir.AxisListType.X, AL.add
            )
            ls = pool.tile([P, NB], F32, tag="ls")
            nc.scalar.activation(ls[:], ssum[:], AF.Ln)
            mls = pool.tile([P, NB], F32, tag="mls")
            nc.vector.tensor_tensor(out=mls[:], in0=mx[:], in1=ls[:], op=AL.add)
            nc.vector.tensor_tensor(
                out=work1[:],
                in0=logits[:, :, :],
                in1=mls[:, :, None].broadcast_to([P, NB, fout]),
                op=AL.subtract,
            )
            for b in range(NB):
                u = used_rows(b)
                nc.sync.dma_start(
                    out=out_sh[b * P : b * P + u, :], in_=work1[:u, b, :]
                )
            nc.leave_named_scope("L3", sc[0], False)

    nc.finalize()
    return nc


def _prepare(x, edge_index, W1, b1, W2, b2, W3, b3, lin_w, lin_b, chunk):
    x = np.asarray(x)
    N = x.shape[0]
    S = N // NC
    fin, fh, fout = W1.shape[0], W2.shape[0], W3.shape[1]
    NB = cdiv(S, P)
    NPAIR = (NB + 1) // 2

    meta = _preprocess(np.asarray(edge_index, dtype=np.int64), N, chunk)
    nc = _build_program(meta, N, chunk, fin, fh, fout)

    iotam = np.tile(
        np.tile(np.arange(P, dtype=np.float16), meta["MAXSL"]), (P, 1)
    )
    ident = np.eye(P, dtype=np.float16)
    lbbc = np.tile(np.asarray(lin_b, np.float16), (P, 1))

    in_maps = []
    for c in range(NC):
        xs = np.asarray(x[c * S : (c + 1) * S], np.float16)
        pc = meta["per_core"][c]
        dis_c = pc["dis"]  # [S] f32

        def pbias(bvec, f):
            # [P, NPAIR, 2*fh]: bias/dis[node] at (tloc, pair, half*fh+feat)
            pb = np.zeros((P, NPAIR, 2 * fh), dtype=np.float32)
            for b in range(NB):
                u = min(P, S - b * P)
                nodes = np.arange(b * P, b * P + u)
                col0 = (b % 2) * fh
                pb[:u, b // 2, col0 : col0 + f] = (
                    bvec[None, :f] / dis_c[nodes][:, None]
                )
            return pb.reshape(P, NPAIR * 2 * fh).astype(np.float16)

        in_maps.append(
            {
                "xT": np.ascontiguousarray(xs.T),
                "W1": np.asarray(W1, np.float16),
                "W2": np.asarray(W2, np.float16),
                "W3": np.asarray(W3, np.float16),
                "LW": np.asarray(lin_w, np.float16),
                "idx16": pc["idx16"],
                "tn": pc["tn"],
                "iotam": iotam,
                "ident": ident,
                "disc": pc["disc"],
                "pb1": pbias(np.asarray(b1, np.float32), fh),
                "pb2": pbias(np.asarray(b2, np.float32), fh),
                "pb3": pbias(np.asarray(b3, np.float32), fout),
                "lbbc": lbbc,
            }
        )
    return nc, in_maps


def kernel(x, edge_index, W1, b1, W2, b2, W3, b3, lin_w, lin_b):
    global LAST_EXEC_NS, LAST_SCOPES
    nc, in_maps = _prepare(
        x, edge_index, W1, b1, W2, b2, W3, b3, lin_w, lin_b, chunk=25000
    )
    trace = bool(os.environ.get("GCN_TRACE"))
    res = run_bass_kernel_spmd(nc, in_maps, list(range(NC)), trace=trace)
    LAST_EXEC_NS = res.exec_time_ns
    LAST_SCOPES = res.per_core_scope_times
    out = np.concatenate([res.results[c]["out_sh"] for c in range(NC)], axis=0)
    return out.astype(np.float32)


# revision 26
# speedup vs baseline: 1.4589x; 1.0014x over previous
"""GCN 3-layer message-passing kernel for TRN2 (8 NeuronCores, SPMD).

Self-contained: takes FULL inputs, shards internally, runs a Bass/Tile
kernel via run_bass_kernel_spmd on cores 0-7, gathers the full output.

v3 strategy (all HW-measured):
  - fp16 tables; grouped dma_gather calls (G=8 target blocks x 4 source
    chunks per layer) with slot-aligned equalized padding so num_idxs is
    a compile-time immediate.
  - GCN norm factored: dis[src] folded into the tables at transform
    evacuation (per-partition scale), dis[dst] applied at the relu
    evacuation; bias/dis[dst] pre-loaded into PSUM by the accumulation
    group's opening matmul. One-hot slot matrices are then pure 0/1 and
    built by a single IS_EQ tensor_tensor per gather call (broadcast AP).
  - log_softmax batched over the whole shard (single Exp / Ln passes) to
    avoid per-block ACT activation-table reloads (~1.3us each).
  - single_packet=False on the gathers (True wedges the device).
"""

import os
import sys

sys.path.insert(0, "/opt/trn_rl_repo")

import numpy as np

import concourse.bass as bass  # noqa: F401
import concourse.mybir as mybir
import concourse.tile as tile
from concourse import bacc
from concourse._compat import cdiv
from concourse.bass_utils import run_bass_kernel_spmd

F32 = mybir.dt.float32
F16 = mybir.dt.float16
I16 = mybir.dt.int16
AL = mybir.AluOpType
AF = mybir.ActivationFunctionType

NC = 8
P = 128
G = 8  # target blocks per gather group

LAST_EXEC_NS = None
LAST_SCOPES = None


def _cdiv_np(a, b):
    return (a + b - 1) // b


def _group_cumcount(grp: np.ndarray) -> np.ndarray:
    n = len(grp)
    if n == 0:
        return np.zeros(0, dtype=np.int64)
    is_new = np.ones(n, dtype=bool)
    is_new[1:] = grp[1:] != grp[:-1]
    idx = np.arange(n)
    start = np.maximum.accumulate(np.where(is_new, idx, 0))
    return idx - start


def _preprocess(edge_index: np.ndarray, n_nodes: int, chunk: int):
    N = n_nodes
    S = N // NC
    NB = cdiv(S, P)
    NG = _cdiv_np(NB, G)
    NQ = 4  # source stripes (one collective each)
    SB = _cdiv_np(NB, NQ)  # blocks per stripe
    srows = [min(SB * P, S - k * SB * P) for k in range(NQ)]
    s_off = np.zeros(NQ, dtype=np.int64)  # stripe start row within a shard

    src = np.concatenate([edge_index[0], np.arange(N, dtype=np.int64)])
    dst = np.concatenate([edge_index[1], np.arange(N, dtype=np.int64)])
    deg = np.bincount(dst, minlength=N).astype(np.float64)
    dis = (1.0 / np.sqrt(deg)).astype(np.float32)

    core = dst // S
    blk = (dst % S) // P
    tloc = (dst % S) % P
    grp = blk // G
    # source stripe mapping: stripe k holds rows [k*SB*P, ...) of every
    # shard; the gathered table t_full{k} is core-major within the stripe
    s_core = src // S
    s_r = src % S
    q = np.minimum(s_r // (SB * P), NQ - 1)
    srows_np = np.array(srows, dtype=np.int64)
    sloc = s_core * srows_np[q] + (s_r - q * SB * P)

    counts = np.zeros((NC, NB, NQ), dtype=np.int64)
    np.add.at(counts, (core, blk, q), 1)
    slots = np.maximum(1, _cdiv_np(counts.max(axis=0), P))  # [NB, NQ] shared

    kb = slots.sum(axis=1)
    K_total = int(kb.sum())
    IW = K_total * 8  # int16 idx columns (128 idxs -> 8 cols of 16)

    slots_gq = np.zeros((NG, NQ), dtype=np.int64)
    sec_off = np.zeros((NB, NQ), dtype=np.int64)  # position offset within call
    for g in range(NG):
        bs = range(g * G, min((g + 1) * G, NB))
        acc = np.zeros(NQ, dtype=np.int64)
        for b in bs:
            sec_off[b, :] = acc * P
            acc += slots[b, :]
        slots_gq[g, :] = acc
    MAXSL = int(slots_gq.max())

    ix_off = np.zeros((NG, NQ), dtype=np.int64)  # idx16 col offset per call
    glob_off = np.zeros((NG, NQ), dtype=np.int64)  # position offset per call
    tn_off = np.zeros((NG, NQ), dtype=np.int64)  # tn col offset per call
    acc = 0
    tacc = 0
    for g in range(NG):
        for qq in range(NQ):
            ix_off[g, qq] = acc // 16
            glob_off[g, qq] = acc
            tn_off[g, qq] = tacc
            acc += int(slots_gq[g, qq]) * P
            tacc += int(slots_gq[g, qq])
    assert acc == K_total * P and acc // 16 == IW and tacc == K_total

    order = np.lexsort((src, blk, q, grp, core))
    so_sloc = sloc[order]
    so_tl = tloc[order]
    so_core = core[order]
    so_blk = blk[order]
    so_q = q[order]
    so_grp = grp[order]

    per_core = []
    for c in range(NC):
        m = so_core == c
        cb, cq, cg = so_blk[m], so_q[m], so_grp[m]
        csl, ctl = so_sloc[m], so_tl[m]
        pos = _group_cumcount(cb * NQ + cq)  # position within (b, q)

        idx_flat = np.zeros(K_total * P, dtype=np.int16)  # dummies -> row 0
        gp = glob_off[cg, cq] + sec_off[cb, cq] + pos
        idx_flat[gp] = csl.astype(np.int16)
        idx16 = np.ascontiguousarray(idx_flat.reshape(-1, 16).T)  # [16, IW]

        tn = np.full((P, K_total), -1.0, dtype=np.float16)  # tv per call slot
        prow = pos % P
        pcol = tn_off[cg, cq] + (sec_off[cb, cq] + pos) // P
        tn[prow, pcol] = ctl.astype(np.float16)

        # per-node dis arranged [tloc, block]; unused rows -> 1.0
        disc = np.ones((P, NB), dtype=np.float32)
        nodes = np.arange(S)
        disc[nodes % P, nodes // P] = dis[c * S : (c + 1) * S]

        per_core.append(
            {"idx16": np.tile(idx16, (8, 1)), "tn": tn, "disc": disc,
             "dis": dis[c * S : (c + 1) * S]}
        )

    return {
        "slots": slots, "slots_gq": slots_gq, "sec_off": sec_off,
        "K_total": K_total, "IW": IW, "MAXSL": MAXSL,
        "ix_off": ix_off, "tn_off": tn_off, "SB": SB,
        "NB": NB, "NG": NG, "NQ": NQ, "per_core": per_core,
    }


def _build_program(meta, n_nodes: int, chunk: int, fin, fh, fout):
    N = n_nodes
    S = N // NC
    NB = meta["NB"]
    NG = meta["NG"]
    NQ = meta["NQ"]
    NPAIR = (NB + 1) // 2
    slots = meta["slots"]
    slots_gq = meta["slots_gq"]
    sec_off = meta["sec_off"]
    K_total = meta["K_total"]
    IW = meta["IW"]
    MAXSL = meta["MAXSL"]
    ix_off = meta["ix_off"]
    tn_off = meta["tn_off"]
    fo_pad = 128  # t3 row padded to 128 fp16 = 256B (gather granularity)

    nc = bacc.Bacc()

    xT = nc.dram_tensor("xT", [fin, S], F16, kind="ExternalInput")
    W1 = nc.dram_tensor("W1", [fin, fh], F16, kind="ExternalInput")
    W2 = nc.dram_tensor("W2", [fh, fh], F16, kind="ExternalInput")
    W3 = nc.dram_tensor("W3", [fh, fout], F16, kind="ExternalInput")
    LW = nc.dram_tensor("LW", [2 * fh + fout, fout], F16, kind="ExternalInput")
    idx16 = nc.dram_tensor("idx16", [P, IW], I16, kind="ExternalInput")
    tn = nc.dram_tensor("tn", [P, K_total], F16, kind="ExternalInput")
    iotam_in = nc.dram_tensor("iotam", [P, MAXSL * P], F16, kind="ExternalInput")
    ident_in = nc.dram_tensor("ident", [P, P], F16, kind="ExternalInput")
    disc_in = nc.dram_tensor("disc", [P, NB], F32, kind="ExternalInput")
    pb1_in = nc.dram_tensor("pb1", [P, NPAIR * 2 * fh], F16, kind="ExternalInput")
    pb2_in = nc.dram_tensor("pb2", [P, NPAIR * 2 * fh], F16, kind="ExternalInput")
    pb3_in = nc.dram_tensor("pb3", [P, NPAIR * 2 * fh], F16, kind="ExternalInput")
    lbbc_in = nc.dram_tensor("lbbc", [P, fout], F16, kind="ExternalInput")
    out_sh = nc.dram_tensor("out_sh", [S, fout], F32, kind="ExternalOutput")

    SB = meta["SB"]
    KS = NQ
    srows = [min(SB * P, S - k * SB * P) for k in range(KS)]
    t1_shs = [nc.dram_tensor(f"t1_sh{k}", [srows[k], fh], F16) for k in range(KS)]
    t2_shs = [nc.dram_tensor(f"t2_sh{k}", [srows[k], fh], F16) for k in range(KS)]
    t3_shs = [
        nc.dram_tensor(f"t3_sh{k}", [srows[k], fo_pad], F16) for k in range(KS)
    ]
    t1_fulls = [
        nc.dram_tensor(f"t1_full{k}", [NC * srows[k], fh], F16,
                       addr_space="Shared")
        for k in range(KS)
    ]
    t2_fulls = [
        nc.dram_tensor(f"t2_full{k}", [NC * srows[k], fh], F16,
                       addr_space="Shared")
        for k in range(KS)
    ]
    t3_fulls = [
        nc.dram_tensor(f"t3_full{k}", [NC * srows[k], fo_pad], F16,
                       addr_space="Shared")
        for k in range(KS)
    ]
    h1T_sh = nc.dram_tensor("h1T_sh", [fh, S], F16)
    h2T_sh = nc.dram_tensor("h2T_sh", [fh, S], F16)

    def stripe_slice(b):
        k = min(b // SB, KS - 1)
        r0 = (b - k * SB) * P
        return k, r0

    rg = [list(range(NC))]

    def used_rows(b):
        return min(P, S - b * P)

    with tile.TileContext(nc) as tc:
        with (
            tc.tile_pool(name="const", bufs=1) as cpool,
            tc.tile_pool(name="sb", bufs=3) as pool,
            tc.tile_pool(name="gath", bufs=2) as gpool,
            tc.tile_pool(name="ps", bufs=1, space="PSUM") as psp,
            tc.tile_pool(name="ps2p", bufs=2, space="PSUM") as psp2,
        ):
            iotam_t = cpool.tile([P, MAXSL * P], F16)
            nc.sync.dma_start(out=iotam_t[:], in_=iotam_in[:, :])
            ident_t = cpool.tile([P, P], F16)
            nc.sync.dma_start(out=ident_t[:], in_=ident_in[:, :])
            disc_t = cpool.tile([P, NB], F32)
            nc.sync.dma_start(out=disc_t[:], in_=disc_in[:, :])
            w1_t = cpool.tile([P, 2, fh], F16)
            nc.sync.dma_start(out=w1_t[:], in_=W1[:, :].rearrange("(c k) f -> k c f", k=P))
            w2_t = cpool.tile([P, 2, fh], F16)
            nc.sync.dma_start(out=w2_t[:], in_=W2[:, :].rearrange("(c k) f -> k c f", k=P))
            w3_t = cpool.tile([P, 2, fout], F16)
            nc.sync.dma_start(out=w3_t[:], in_=W3[:, :].rearrange("(c k) f -> k c f", k=P))
            lw12_t = cpool.tile([P, 4, fout], F16)
            nc.sync.dma_start(
                out=lw12_t[:], in_=LW[: 4 * P, :].rearrange("(c k) f -> k c f", k=P)
            )
            lw3_t = cpool.tile([fout, fout], F16)
            nc.sync.dma_start(out=lw3_t[:], in_=LW[4 * P :, :])
            lbbc = cpool.tile([P, fout], F16)
            nc.sync.dma_start(out=lbbc[:], in_=lbbc_in[:, :])
            idxsb = cpool.tile([P, IW], I16)
            nc.sync.dma_start(out=idxsb[:], in_=idx16[:, :])
            tnsb = cpool.tile([P, K_total], F16)
            nc.sync.dma_start(out=tnsb[:], in_=tn[:, :])
            logits = cpool.tile([P, NB, fout], F32)
            work1 = cpool.tile([P, NB, fout], F32)
            work2 = cpool.tile([P, NB, fout], F32)

            sc_T = nc.enter_named_scope("phaseT", False)
            for b in range(NB):
                u = used_rows(b)
                ps1 = psp2.tile([P, fh], F32, tag="ps2")
                for cc in range(2):
                    xt = pool.tile([P, P], F16, tag="xt")
                    nc.sync.dma_start(
                        out=xt[:, :u], in_=xT[cc * P : (cc + 1) * P, b * P : b * P + u]
                    )
                    nc.tensor.matmul(
                        ps1[:u, :],
                        lhsT=xt[:, :u],
                        rhs=w1_t[:, cc, :],
                        start=(cc == 0),
                        stop=(cc == 1),
                    )
                ev = pool.tile([P, fh], F16, tag="ev")
                nc.vector.tensor_scalar(
                    out=ev[:u, :], in0=ps1[:u, :], scalar1=disc_t[:u, b : b + 1],
                    scalar2=None, op0=AL.mult,
                )
                k, r0 = stripe_slice(b)
                nc.sync.dma_start(out=t1_shs[k][r0 : r0 + u, :], in_=ev[:u, :])
                if b == min((k + 1) * SB, NB) - 1:
                    nc.gpsimd.collective_compute(
                        "AllGather", AL.bypass, ins=[t1_shs[k][:, :]],
                        outs=[t1_fulls[k][:, :]], replica_groups=rg,
                    )

            nc.leave_named_scope("phaseT", sc_T[0], False)

            def layer(li, tables, felem, wnext_t, fnext, pb_in, tnext_shs,
                      tnext_fulls, hT_sh_, fnext_pad=None):
                fagg = fh if li < 3 else fout
                for g in range(NG):
                    b_lo = g * G
                    b_hi = min((g + 1) * G, NB)
                    nb_g = b_hi - b_lo
                    npair_g = (nb_g + 1) // 2

                    psabs = []
                    for i in range(npair_g):
                        pbt = pool.tile([P, 2 * fh], F16, tag="pbt", name=f"pbt{i}")
                        po = (b_lo // 2 + i) * 2 * fh
                        nc.scalar.dma_start(
                            out=pbt[:], in_=pb_in[:, po : po + 2 * fh]
                        )
                        pt = psp.tile([P, 2 * fh], F32, tag=f"psab{i}",
                                      name=f"psab{i}")
                        nc.tensor.matmul(
                            pt[:], lhsT=ident_t[:], rhs=pbt[:],
                            start=True, stop=False,
                        )
                        psabs.append(pt)
                    psas = [
                        psabs[i // 2][:, (i % 2) * fh : (i % 2 + 1) * fh]
                        for i in range(nb_g)
                    ]

                    for qq in range(NQ):
                        sl_gq = int(slots_gq[g, qq])
                        nidx = sl_gq * P
                        io = int(ix_off[g, qq])
                        tno = int(tn_off[g, qq])
                        dst = gpool.tile([P, MAXSL, felem], F16, tag="dst")
                        rows = NC * srows[qq]
                        nc.gpsimd.dma_gather(
                            dst[:, :sl_gq, :],
                            tables[qq][0:rows, :],
                            idxsb[:, io : io + sl_gq * 8],
                            nidx,
                            nidx,
                            felem,
                            single_packet=False,
                        )
                        stb = gpool.tile([P, MAXSL, P], F16, tag="stb")
                        tv_b = tnsb[:, tno : tno + sl_gq, None].broadcast_to(
                            [P, sl_gq, P]
                        )
                        nc.vector.tensor_tensor(
                            out=stb[:, :sl_gq, :],
                            in0=iotam_t[:, : sl_gq * P].rearrange(
                                "p (s c) -> p s c", c=P
                            ),
                            in1=tv_b,
                            op=AL.is_equal,
                        )
                        for bi in range(nb_g):
                            b = b_lo + bi
                            col0 = int(sec_off[b, qq]) // P
                            closer = (bi % 2 == 1) or (bi == nb_g - 1)
                            nsl = int(slots[b, qq])
                            for j in range(nsl):
                                s = col0 + j
                                last = qq == NQ - 1 and closer and j == nsl - 1
                                nc.tensor.matmul(
                                    psas[bi][:, :fagg],
                                    lhsT=stb[:, s, :],
                                    rhs=dst[:, s, :fagg],
                                    start=False,
                                    stop=last,
                                )

                    for bi in range(nb_g):
                        b = b_lo + bi
                        u = used_rows(b)
                        psa = psas[bi]
                        h_sb = pool.tile([P, fagg], F16, tag="h_sb")
                        nc.scalar.activation(
                            h_sb[:], psa[:, :fagg], AF.Relu,
                            scale=disc_t[:, b : b + 1],
                        )

                        if li < 3:
                            ps2 = psp2.tile([P, fnext], F32, tag="ps2")
                            for cc in range(2):
                                pst = psp2.tile([P, P], F16, tag="pst")
                                nc.tensor.transpose(
                                    pst[:], h_sb[:, cc * P : (cc + 1) * P], ident_t[:]
                                )
                                hT = pool.tile([P, P], F16, tag="hT")
                                nc.vector.tensor_copy(hT[:], pst[:])
                                nc.sync.dma_start(
                                    out=hT_sh_[cc * P : (cc + 1) * P, b * P : b * P + u],
                                    in_=hT[:, :u],
                                )
                                nc.tensor.matmul(
                                    ps2[:u, :],
                                    lhsT=hT[:, :u],
                                    rhs=wnext_t[:, cc, :fnext],
                                    start=(cc == 0),
                                    stop=(cc == 1),
                                )
                            fp = fnext_pad or fnext
                            ev2 = pool.tile([P, fp], F16, tag="ev")
                            nc.vector.tensor_scalar(
                                out=ev2[:u, :fnext], in0=ps2[:u, :],
                                scalar1=disc_t[:u, b : b + 1],
                                scalar2=None, op0=AL.mult,
                            )
                            if fp > fnext:
                                nc.vector.memset(ev2[:u, fnext:fp], 0.0)
                            k, r0 = stripe_slice(b)
                            nc.sync.dma_start(
                                out=tnext_shs[k][r0 : r0 + u, :fp],
                                in_=ev2[:u, :fp],
                            )
                            if b == min((k + 1) * SB, NB) - 1:
                                nc.gpsimd.collective_compute(
                                    "AllGather", AL.bypass,
                                    ins=[tnext_shs[k][:, :]],
                                    outs=[tnext_fulls[k][:, :]],
                                    replica_groups=rg,
                                )
                        else:
                            ps3t = psp2.tile([fout, P], F16, tag="pst")
                            nc.tensor.transpose(ps3t[:], h_sb[:, :fout], ident_t[:])
                            h3T = pool.tile([fout, P], F16, tag="hT")
                            nc.vector.tensor_copy(h3T[:], ps3t[:])
                            pso = psp2.tile([P, fout], F32, tag="ps2")
                            for cc in range(2):
                                r1 = pool.tile([P, P], F16, tag=f"rl{cc}")
                                nc.scalar.dma_start(
                                    out=r1[:, :u],
                                    in_=h1T_sh[cc * P : (cc + 1) * P, b * P : b * P + u],
                                )
                                nc.tensor.matmul(
                                    pso[:u, :], lhsT=r1[:, :u], rhs=lw12_t[:, cc, :],
                                    start=(cc == 0), stop=False,
                                )
                            for cc in range(2):
                                r2 = pool.tile([P, P], F16, tag=f"rl{2 + cc}")
                                nc.scalar.dma_start(
                                    out=r2[:, :u],
                                    in_=h2T_sh[cc * P : (cc + 1) * P, b * P : b * P + u],
                                )
                                nc.tensor.matmul(
                                    pso[:u, :], lhsT=r2[:, :u], rhs=lw12_t[:, 2 + cc, :],
                                    start=False, stop=False,
                                )
                            nc.tensor.matmul(
                                pso[:u, :], lhsT=h3T[:, :u], rhs=lw3_t[:, :],
                                start=False, stop=False,
                            )
                            nc.tensor.matmul(
                                pso[:u, :], lhsT=ident_t[:, :u], rhs=lbbc[:, :],
                                start=False, stop=True,
                            )
                            nc.vector.tensor_copy(logits[:, b, :], pso[:, :])

            sc = nc.enter_named_scope("L1", False)
            layer(1, t1_fulls, fh, w2_t, fh, pb1_in, t2_shs, t2_fulls, h1T_sh)
            nc.leave_named_scope("L1", sc[0], False)
            sc = nc.enter_named_scope("L2", False)
            layer(2, t2_fulls, fh, w3_t, fout, pb2_in, t3_shs, t3_fulls,
                  h2T_sh, fnext_pad=fo_pad)
            nc.leave_named_scope("L2", sc[0], False)
            sc = nc.enter_named_scope("L3", False)
            layer(3, t3_fulls, fo_pad, None, None, pb3_in, None, None, None)

            # batched log_softmax over the whole shard
            mx = pool.tile([P, NB], F32, tag="mx")
            nc.vector.tensor_reduce(
                mx[:], logits[:, :, :], mybir.AxisListType.X, AL.max
            )
            nc.vector.tensor_tensor(
                out=work1[:],
                in0=logits[:, :, :],
                in1=mx[:, :, None].broadcast_to([P, NB, fout]),
                op=AL.subtract,
            )
            nc.scalar.activation(
                work2[:].rearrange("p a b -> p (a b)"),
                work1[:].rearrange("p a b -> p (a b)"),
                AF.Exp,
            )
            ssum = pool.tile([P, NB], F32, tag="ssum")
            nc.vector.tensor_reduce(
                ssum[:], work2[:, :, :], mybir.AxisListType.X, AL.add
            )
            ls = pool.tile([P, NB], F32, tag="ls")
            nc.scalar.activation(ls[:], ssum[:], AF.Ln)
            mls = pool.tile([P, NB], F32, tag="mls")
            nc.vector.tensor_tensor(out=mls[:], in0=mx[:], in1=ls[:], op=AL.add)
            nc.vector.tensor_tensor(
                out=work1[:],
                in0=logits[:, :, :],
                in1=mls[:, :, None].broadcast_to([P, NB, fout]),
                op=AL.subtract,
            )
            for b in range(NB):
                u = used_rows(b)
                nc.sync.dma_start(
                    out=out_sh[b * P : b * P + u, :], in_=work1[:u, b, :]
                )
            nc.leave_named_scope("L3", sc[0], False)

    nc.finalize()
    return nc


def _prepare(x, edge_index, W1, b1, W2, b2, W3, b3, lin_w, lin_b, chunk):
    x = np.asarray(x)
    N = x.shape[0]
    S = N // NC
    fin, fh, fout = W1.shape[0], W2.shape[0], W3.shape[1]
    NB = cdiv(S, P)
    NPAIR = (NB + 1) // 2

    meta = _preprocess(np.asarray(edge_index, dtype=np.int64), N, chunk)
    nc = _build_program(meta, N, chunk, fin, fh, fout)

    iotam = np.tile(
        np.tile(np.arange(P, dtype=np.float16), meta["MAXSL"]), (P, 1)
    )
    ident = np.eye(P, dtype=np.float16)
    lbbc = np.tile(np.asarray(lin_b, np.float16), (P, 1))

    in_maps = []
    for c in range(NC):
        xs = np.asarray(x[c * S : (c + 1) * S], np.float16)
        pc = meta["per_core"][c]
        dis_c = pc["dis"]  # [S] f32

        def pbias(bvec, f):
            # [P, NPAIR, 2*fh]: bias/dis[node] at (tloc, pair, half*fh+feat)
            pb = np.zeros((P, NPAIR, 2 * fh), dtype=np.float32)
            for b in range(NB):
                u = min(P, S - b * P)
                nodes = np.arange(b * P, b * P + u)
                col0 = (b % 2) * fh
                pb[:u, b // 2, col0 : col0 + f] = (
                    bvec[None, :f] / dis_c[nodes][:, None]
                )
            return pb.reshape(P, NPAIR * 2 * fh).astype(np.float16)

        in_maps.append(
            {
                "xT": np.ascontiguousarray(xs.T),
                "W1": np.asarray(W1, np.float16),
                "W2": np.asarray(W2, np.float16),
                "W3": np.asarray(W3, np.float16),
                "LW": np.asarray(lin_w, np.float16),
                "idx16": pc["idx16"],
                "tn": pc["tn"],
                "iotam": iotam,
                "ident": ident,
                "disc": pc["disc"],
                "pb1": pbias(np.asarray(b1, np.float32), fh),
                "pb2": pbias(np.asarray(b2, np.float32), fh),
                "pb3": pbias(np.asarray(b3, np.float32), fout),
                "lbbc": lbbc,
            }
        )
    return nc, in_maps


def kernel(x, edge_index, W1, b1, W2, b2, W3, b3, lin_w, lin_b):
    global LAST_EXEC_NS, LAST_SCOPES
    nc, in_maps = _prepare(
        x, edge_index, W1, b1, W2, b2, W3, b3, lin_w, lin_b, chunk=25000
    )
    trace = bool(os.environ.get("GCN_TRACE"))
    res = run_bass_kernel_spmd(nc, in_maps, list(range(NC)), trace=trace)
    LAST_EXEC_NS = res.exec_time_ns
    LAST_SCOPES = res.per_core_scope_times
    out = np.concatenate([res.results[c]["out_sh"] for c in range(NC)], axis=0)
    return out.astype(np.float32)
